# revision 5
# baseline (speedup 1.0000x reference)
"""Trainium2 Bass kernel for GroupLinear:
    out = einsum('lgi,lgj,ogij->lo', x1, x2, W.reshape(O,g,b,b)) + bias

Equivalent to Y = outer @ W.T + b where outer[l, k] (k = g*b*b + i*b + j) is
the blockwise outer product x1[l,g,i]*x2[l,g,j] -- a [2048, 65536] @
[65536, 1024] matmul whose LHS is generated on the fly.

Sharding: tensor-parallel over the contraction dim. Core c owns weight
blocks g in {2c, 2c+1} (K_local = 8192), computes a full [2048, 1024]
partial, and the host sums the 8 partials (+ bias). The replicated-x1
operand layout is prepared host-side (a pure gather -- no FLOPs); the
outer products themselves and all matmul FLOPs run on device (DVE + PE).

Schedule (per core): the kernel is PE-bound (~437us of matmul streaming),
so the schedule exists to keep the PE dense from ~9us on:
  - Phase 1 (l-blocks 0-2, chunk-outer): 6 psum tiles accumulate three
    l-blocks at once, so each W chunk is consumed 6 matmuls at a time as
    it arrives -- the PE's W demand rate (~280 GB/s) stays under the DMA
    supply rate while the 16.75 MB weight shard streams in.
  - Phase 2 (l-blocks 3-15, lb-outer): W is SBUF-resident; per chunk
    LDW + 2 matmuls (N=512) into a rotating set of psum banks.
  - Startup DMAs are split fine-grained and spread over both HWDGE
    queues (sync + scalar) so the first matmul issues as early as
    possible; output is bf16 to shrink the drain tail.
"""

import sys
import numpy as np

sys.path.insert(0, "/opt/trn_rl_repo")

import ml_dtypes  # noqa: E402

BF16 = ml_dtypes.bfloat16

L = 2048
H = 1024
O = 1024
B = 64
G = 16
NCORES = 8
GPC = G // NCORES          # weight blocks per core = 2
KL = GPC * B * B           # local contraction dim = 8192
NCHUNK = KL // 128         # 64 k-chunks of 128
LB = 128                   # l-block (tokens per psum tile)
NLB = L // LB              # 16
R1 = 3                     # l-blocks processed chunk-outer in phase 1

_cache = {}


def _build_nc():
    from concourse import bass, tile, bacc

    mybir = bass.mybir
    bf = mybir.dt.bfloat16
    f32 = mybir.dt.float32

    nc = bacc.Bacc("TRN2", target_bir_lowering=False, debug=False)
    wp = nc.dram_tensor("wp", [128, NCHUNK * O], bf, kind="ExternalInput")
    x1r = nc.dram_tensor("x1r", [NLB, 128, KL], bf, kind="ExternalInput")
    x2s = nc.dram_tensor("x2s", [GPC, 128, L], bf, kind="ExternalInput")
    out = nc.dram_tensor("out", [L, O], bf, kind="ExternalOutput")

    XA = R1 * LB               # x2 columns needed by phase 1 = 384

    with tile.TileContext(nc) as tc:
        with (
            tc.tile_pool(name="wpool", bufs=1) as wpool,
            tc.tile_pool(name="x2pool", bufs=1) as x2pool,
            tc.tile_pool(name="xpool", bufs=4) as xpool,
            tc.tile_pool(name="opool", bufs=2) as opool,
            tc.tile_pool(name="psum", bufs=2, space="PSUM") as psum,
        ):
            wt = wpool.tile([128, NCHUNK * O], bf)
            x2a = [x2pool.tile([128, XA], bf, tag=f"x2a_{g}", name=f"x2a_{g}") for g in range(GPC)]
            x2b = [x2pool.tile([128, L - XA], bf, tag=f"x2b_{g}", name=f"x2b_{g}") for g in range(GPC)]
            xts = [xpool.tile([128, KL], bf, tag="xt", name=f"xt_{r}") for r in range(R1)]

            def wpiece(c0, c1, eng):
                sl = slice(c0 * O, c1 * O)
                eng.dma_start(wt[:, sl], wp[:, sl])

            # startup: smallest, soonest-needed pieces first. sync carries
            # the weight stream; scalar carries x2 + phase-1 slab pieces.
            wpiece(0, 1, nc.sync)
            for g in range(GPC):
                nc.scalar.dma_start(x2a[g][:], x2s[g][:, 0:XA])
            for r in range(R1):
                nc.scalar.dma_start(xts[r][:, 0:512], x1r[r][:, 0:512])
            wpiece(1, 4, nc.sync)
            for r in range(R1):
                nc.scalar.dma_start(xts[r][:, 512:2048], x1r[r][:, 512:2048])
            wpiece(4, 8, nc.sync)
            wpiece(8, 16, nc.sync)
            for r in range(R1):
                nc.scalar.dma_start(xts[r][:, 2048:KL], x1r[r][:, 2048:KL])
            for g in range(GPC):
                nc.scalar.dma_start(x2b[g][:], x2s[g][:, XA:L])
            wpiece(16, 32, nc.sync)
            wpiece(32, 64, nc.sync)

            # ---- phase 1: l-blocks 0..R1-1, chunk-outer ----
            ps1 = [
                [psum.tile([128, 512], f32, name=f"psA{r}{h}", tag=f"psA{r}{h}",
                           bufs=1) for h in range(2)]
                for r in range(R1)
            ]
            for c in range(NCHUNK):
                g = c >> 5
                csl = slice(c * 128, (c + 1) * 128)
                first, last = c == 0, c == NCHUNK - 1
                for r in range(R1):
                    nc.vector.tensor_mul(
                        xts[r][:, csl], xts[r][:, csl],
                        x2a[g][:, r * LB:(r + 1) * LB],
                    )
                for r in range(R1):
                    for h in range(2):
                        nc.tensor.matmul(
                            ps1[r][h][:],
                            xts[r][:, csl],
                            wt[:, c * O + h * 512:c * O + (h + 1) * 512],
                            start=first,
                            stop=last,
                        )
            for r in range(R1):
                ot = opool.tile([128, O], bf, tag="ot", name="ot")
                nc.scalar.mul(ot[:, 0:512], ps1[r][0][:], 1.0)
                nc.scalar.mul(ot[:, 512:O], ps1[r][1][:], 1.0)
                nc.sync.dma_start(out[r * LB:(r + 1) * LB, :], ot[:])

            # ---- phase 2: l-blocks R1..NLB-1, lb-outer, two-pass over the
            # psum halves (ps0's drain overlaps ps1's matmul pass) ----
            for lb in range(R1, NLB):
                xt = xpool.tile([128, KL], bf, tag="xt", name="xt")
                nc.scalar.dma_start(xt[:], x1r[lb])
                lsl = slice(lb * LB, (lb + 1) * LB)
                bsl = slice(lb * LB - XA, (lb + 1) * LB - XA)
                ps0 = psum.tile([128, 512], f32, tag="ps", name="ps0")
                ps1_ = psum.tile([128, 512], f32, tag="ps", name="ps1")
                for c in range(NCHUNK):
                    g = c >> 5
                    csl = slice(c * 128, (c + 1) * 128)
                    nc.vector.tensor_mul(xt[:, csl], xt[:, csl], x2b[g][:, bsl])
                    nc.tensor.matmul(
                        ps0[:], xt[:, csl], wt[:, c * O:c * O + 512],
                        start=(c == 0), stop=(c == NCHUNK - 1),
                    )
                ot = opool.tile([128, O], bf, tag="ot", name="ot")
                nc.scalar.mul(ot[:, 0:512], ps0[:], 1.0)
                if lb == NLB - 1:
                    nc.sync.dma_start(out[lsl, 0:512], ot[:, 0:512])
                for c in range(NCHUNK):
                    csl = slice(c * 128, (c + 1) * 128)
                    nc.tensor.matmul(
                        ps1_[:], xt[:, csl], wt[:, c * O + 512:(c + 1) * O],
                        start=(c == 0), stop=(c == NCHUNK - 1),
                    )
                nc.scalar.mul(ot[:, 512:O], ps1_[:], 1.0)
                if lb == NLB - 1:
                    nc.sync.dma_start(out[lsl, 512:O], ot[:, 512:O])
                else:
                    nc.sync.dma_start(out[lsl, :], ot[:])

    nc.compile()
    return nc


def _prep_inputs(input1, input2, W):
    """Host-side shard + layout (transposes / gathers / dtype casts only)."""
    x1 = np.ascontiguousarray(input1, dtype=np.float32)
    x2 = np.ascontiguousarray(input2, dtype=np.float32)
    Wt = np.ascontiguousarray(W.T, dtype=np.float32)  # [65536, 1024], k-major

    in_maps = []
    for core in range(NCORES):
        ks = slice(core * KL, (core + 1) * KL)
        gs = slice(core * GPC, (core + 1) * GPC)
        # weights: [k_local, o] -> [c, p, o] -> [p, c*O + o]
        wp = (
            Wt[ks]
            .reshape(NCHUNK, 128, O)
            .transpose(1, 0, 2)
            .reshape(128, NCHUNK * O)
            .astype(BF16)
        )
        # x1 replicated over j: k_local = g*B*B + i*B + j -> x1[l, g, i]
        x1g = x1.reshape(L, G, B)[:, gs, :].transpose(1, 2, 0)  # [g, i, l]
        rep = np.repeat(x1g, B, axis=1).reshape(KL, L)          # [k_local, l]
        x1r = (
            rep.reshape(NCHUNK, 128, NLB, LB)
            .transpose(2, 1, 0, 3)
            .reshape(NLB, 128, KL)
            .astype(BF16)
        )
        # x2 stacked twice along partitions: row p -> j = p % 64
        x2g = x2.reshape(L, G, B)[:, gs, :].transpose(1, 2, 0)  # [g, j, l]
        x2st = np.concatenate([x2g, x2g], axis=1).astype(BF16)  # [g, 128, l]
        in_maps.append(
            {
                "wp": np.ascontiguousarray(wp),
                "x1r": np.ascontiguousarray(x1r),
                "x2s": np.ascontiguousarray(x2st),
            }
        )
    return in_maps


def run(input1, input2, W, b, trace=False, tmpdir=None):
    """Shard, run on 8 NeuronCores, unshard. Returns (out, BassKernelResults)."""
    from concourse.bass_utils import run_bass_kernel_spmd

    if "nc" not in _cache:
        _cache["nc"] = _build_nc()
    nc = _cache["nc"]

    in_maps = _prep_inputs(input1, input2, W)
    res = run_bass_kernel_spmd(
        nc, in_maps, list(range(NCORES)), trace=trace, tmpdir=tmpdir
    )
    acc = np.zeros((L, O), dtype=np.float32)
    for core in range(NCORES):
        acc += res.results[core]["out"].astype(np.float32)
    acc += np.asarray(b, dtype=np.float32)[None, :]
    return acc, res


def kernel(input1, input2, W, b):
    out, _ = run(input1, input2, W, b, trace=False)
    return out


if __name__ == "__main__":
    rng = np.random.default_rng(0)
    x1 = rng.standard_normal((L, H), dtype=np.float32)
    x2 = rng.standard_normal((L, H), dtype=np.float32)
    W = rng.standard_normal((O, H * B), dtype=np.float32) / 256.0
    b = rng.standard_normal((O,), dtype=np.float32) / 256.0
    out = kernel(x1, x2, W, b)
    print("out", out.shape, out.dtype, float(np.abs(out).max()))


# revision 6
# speedup vs baseline: 1.0670x; 1.0670x over previous
"""Trainium2 Bass kernel for GroupLinear:
    out = einsum('lgi,lgj,ogij->lo', x1, x2, W.reshape(O,g,b,b)) + bias

Equivalent to Y = outer @ W.T + b where outer[l, k] (k = g*b*b + i*b + j) is
the blockwise outer product x1[l,g,i]*x2[l,g,j] -- a [2048, 65536] @
[65536, 1024] matmul whose LHS is generated on the fly.

Sharding: tensor-parallel over the contraction dim. Core c owns weight
blocks g in {2c, 2c+1} (K_local = 8192), computes a full [2048, 1024]
partial, and the host sums the 8 partials (+ bias). The replicated-x1
operand layout is prepared host-side (a pure gather -- no FLOPs); the
outer products themselves and all matmul FLOPs run on device (DVE + PE).

Schedule (per core): the kernel is PE-bound (~437us of matmul streaming),
so the schedule exists to keep the PE dense from ~9us on:
  - Phase 1 (l-blocks 0-2, chunk-outer): 6 psum tiles accumulate three
    l-blocks at once, so each W chunk is consumed 6 matmuls at a time as
    it arrives -- the PE's W demand rate (~280 GB/s) stays under the DMA
    supply rate while the 16.75 MB weight shard streams in.
  - Phase 2 (l-blocks 3-15, lb-outer): W is SBUF-resident; per chunk
    LDW + 2 matmuls (N=512) into a rotating set of psum banks.
  - Startup DMAs are split fine-grained and spread over both HWDGE
    queues (sync + scalar) so the first matmul issues as early as
    possible; output is bf16 to shrink the drain tail.
"""

import sys
import numpy as np

sys.path.insert(0, "/opt/trn_rl_repo")

import ml_dtypes  # noqa: E402

BF16 = ml_dtypes.bfloat16

L = 2048
H = 1024
O = 1024
B = 64
G = 16
NCORES = 8
GPC = G // NCORES          # weight blocks per core = 2
KL = GPC * B * B           # local contraction dim = 8192
NCHUNK = KL // 128         # 64 k-chunks of 128
LB = 128                   # l-block (tokens per psum tile)
NLB = L // LB              # 16
R1 = 3                     # l-blocks processed chunk-outer in phase 1

_cache = {}


def _build_nc():
    from concourse import bass, tile, bacc

    mybir = bass.mybir
    bf = mybir.dt.bfloat16
    f32 = mybir.dt.float32

    nc = bacc.Bacc("TRN2", target_bir_lowering=False, debug=False)
    wp = nc.dram_tensor("wp", [128, NCHUNK * O], bf, kind="ExternalInput")
    x1r = nc.dram_tensor("x1r", [NLB, 128, KL], bf, kind="ExternalInput")
    x2s = nc.dram_tensor("x2s", [GPC, 128, L], bf, kind="ExternalInput")
    out = nc.dram_tensor("out", [L, O], bf, kind="ExternalOutput")

    XA = R1 * LB               # x2 columns needed by phase 1 = 384

    with tile.TileContext(nc) as tc:
        with (
            tc.tile_pool(name="wpool", bufs=1) as wpool,
            tc.tile_pool(name="x2pool", bufs=1) as x2pool,
            tc.tile_pool(name="xpool", bufs=4) as xpool,
            tc.tile_pool(name="opool", bufs=2) as opool,
            tc.tile_pool(name="psum", bufs=2, space="PSUM") as psum,
        ):
            wt = wpool.tile([128, NCHUNK * O], bf)
            x2a = [x2pool.tile([128, XA], bf, tag=f"x2a_{g}", name=f"x2a_{g}") for g in range(GPC)]
            x2b = [x2pool.tile([128, L - XA], bf, tag=f"x2b_{g}", name=f"x2b_{g}") for g in range(GPC)]
            xts = [xpool.tile([128, KL], bf, tag="xt", name=f"xt_{r}") for r in range(R1)]

            def wpiece(c0, c1, eng):
                sl = slice(c0 * O, c1 * O)
                eng.dma_start(wt[:, sl], wp[:, sl])

            # startup streams, in consumption order, small pieces first.
            # sync carries the weight stream (semaphore granularity must
            # track the PE's chunk-by-chunk consumption in phase 1);
            # scalar carries x2 + the phase-1 slab pieces.
            W_PIECES = [(0, 1), (1, 2), (2, 3), (3, 4), (4, 6), (6, 8)] + [
                (a, a + 4) for a in range(8, NCHUNK, 4)
            ]
            SLAB_PIECES = [(0, 1024), (1024, 3072), (3072, 6144), (6144, KL)]
            for c0, c1 in W_PIECES:
                wpiece(c0, c1, nc.sync)
            for g in range(GPC):
                nc.scalar.dma_start(x2a[g][:], x2s[g][:, 0:XA])
            for a, b_ in SLAB_PIECES:
                for r in range(R1):
                    nc.scalar.dma_start(xts[r][:, a:b_], x1r[r][:, a:b_])
            for g in range(GPC):
                nc.scalar.dma_start(x2b[g][:], x2s[g][:, XA:L])

            # ---- phase 1: l-blocks 0..R1-1, chunk-outer ----
            ps1 = [
                [psum.tile([128, 512], f32, name=f"psA{r}{h}", tag=f"psA{r}{h}",
                           bufs=1) for h in range(2)]
                for r in range(R1)
            ]
            for c in range(NCHUNK):
                g = c >> 5
                csl = slice(c * 128, (c + 1) * 128)
                first, last = c == 0, c == NCHUNK - 1
                for r in range(R1):
                    nc.vector.tensor_mul(
                        xts[r][:, csl], xts[r][:, csl],
                        x2a[g][:, r * LB:(r + 1) * LB],
                    )
                for r in range(R1):
                    for h in range(2):
                        nc.tensor.matmul(
                            ps1[r][h][:],
                            xts[r][:, csl],
                            wt[:, c * O + h * 512:c * O + (h + 1) * 512],
                            start=first,
                            stop=last,
                        )
            xt_next = xpool.tile([128, KL], bf, tag="xt", name="xt")
            nc.scalar.dma_start(xt_next[:], x1r[R1])

            for r in range(R1):
                ot = opool.tile([128, O], bf, tag="ot", name="ot")
                nc.scalar.mul(ot[:, 0:512], ps1[r][0][:], 1.0)
                nc.scalar.mul(ot[:, 512:O], ps1[r][1][:], 1.0)
                nc.sync.dma_start(out[r * LB:(r + 1) * LB, :], ot[:])

            # ---- phase 2: l-blocks R1..NLB-1, lb-outer, two-pass over the
            # psum halves (ps0's drain overlaps ps1's matmul pass) ----
            for lb in range(R1, NLB):
                xt = xt_next
                if lb + 1 < NLB:
                    xt_next = xpool.tile([128, KL], bf, tag="xt", name="xt")
                    nc.scalar.dma_start(xt_next[:], x1r[lb + 1])
                lsl = slice(lb * LB, (lb + 1) * LB)
                bsl = slice(lb * LB - XA, (lb + 1) * LB - XA)
                ps0 = psum.tile([128, 512], f32, tag="ps", name="ps0")
                ps1_ = psum.tile([128, 512], f32, tag="ps", name="ps1")
                for c in range(NCHUNK):
                    g = c >> 5
                    csl = slice(c * 128, (c + 1) * 128)
                    nc.vector.tensor_mul(xt[:, csl], xt[:, csl], x2b[g][:, bsl])
                    nc.tensor.matmul(
                        ps0[:], xt[:, csl], wt[:, c * O:c * O + 512],
                        start=(c == 0), stop=(c == NCHUNK - 1),
                    )
                ot = opool.tile([128, O], bf, tag="ot", name="ot")
                nc.scalar.mul(ot[:, 0:512], ps0[:], 1.0)
                if lb == NLB - 1:
                    nc.sync.dma_start(out[lsl, 0:512], ot[:, 0:512])
                for c in range(NCHUNK):
                    csl = slice(c * 128, (c + 1) * 128)
                    nc.tensor.matmul(
                        ps1_[:], xt[:, csl], wt[:, c * O + 512:(c + 1) * O],
                        start=(c == 0), stop=(c == NCHUNK - 1),
                    )
                nc.scalar.mul(ot[:, 512:O], ps1_[:], 1.0)
                if lb == NLB - 1:
                    nc.sync.dma_start(out[lsl, 512:O], ot[:, 512:O])
                else:
                    nc.sync.dma_start(out[lsl, :], ot[:])

    nc.compile()
    return nc


def _prep_inputs(input1, input2, W):
    """Host-side shard + layout (transposes / gathers / dtype casts only)."""
    x1 = np.ascontiguousarray(input1, dtype=np.float32)
    x2 = np.ascontiguousarray(input2, dtype=np.float32)
    Wt = np.ascontiguousarray(W.T, dtype=np.float32)  # [65536, 1024], k-major

    in_maps = []
    for core in range(NCORES):
        ks = slice(core * KL, (core + 1) * KL)
        gs = slice(core * GPC, (core + 1) * GPC)
        # weights: [k_local, o] -> [c, p, o] -> [p, c*O + o]
        wp = (
            Wt[ks]
            .reshape(NCHUNK, 128, O)
            .transpose(1, 0, 2)
            .reshape(128, NCHUNK * O)
            .astype(BF16)
        )
        # x1 replicated over j: k_local = g*B*B + i*B + j -> x1[l, g, i]
        x1g = x1.reshape(L, G, B)[:, gs, :].transpose(1, 2, 0)  # [g, i, l]
        rep = np.repeat(x1g, B, axis=1).reshape(KL, L)          # [k_local, l]
        x1r = (
            rep.reshape(NCHUNK, 128, NLB, LB)
            .transpose(2, 1, 0, 3)
            .reshape(NLB, 128, KL)
            .astype(BF16)
        )
        # x2 stacked twice along partitions: row p -> j = p % 64
        x2g = x2.reshape(L, G, B)[:, gs, :].transpose(1, 2, 0)  # [g, j, l]
        x2st = np.concatenate([x2g, x2g], axis=1).astype(BF16)  # [g, 128, l]
        in_maps.append(
            {
                "wp": np.ascontiguousarray(wp),
                "x1r": np.ascontiguousarray(x1r),
                "x2s": np.ascontiguousarray(x2st),
            }
        )
    return in_maps


def run(input1, input2, W, b, trace=False, tmpdir=None):
    """Shard, run on 8 NeuronCores, unshard. Returns (out, BassKernelResults)."""
    from concourse.bass_utils import run_bass_kernel_spmd

    if "nc" not in _cache:
        _cache["nc"] = _build_nc()
    nc = _cache["nc"]

    in_maps = _prep_inputs(input1, input2, W)
    res = run_bass_kernel_spmd(
        nc, in_maps, list(range(NCORES)), trace=trace, tmpdir=tmpdir
    )
    acc = np.zeros((L, O), dtype=np.float32)
    for core in range(NCORES):
        acc += res.results[core]["out"].astype(np.float32)
    acc += np.asarray(b, dtype=np.float32)[None, :]
    return acc, res


def kernel(input1, input2, W, b):
    out, _ = run(input1, input2, W, b, trace=False)
    return out


if __name__ == "__main__":
    rng = np.random.default_rng(0)
    x1 = rng.standard_normal((L, H), dtype=np.float32)
    x2 = rng.standard_normal((L, H), dtype=np.float32)
    W = rng.standard_normal((O, H * B), dtype=np.float32) / 256.0
    b = rng.standard_normal((O,), dtype=np.float32) / 256.0
    out = kernel(x1, x2, W, b)
    print("out", out.shape, out.dtype, float(np.abs(out).max()))


# revision 8
# speedup vs baseline: 1.0711x; 1.0039x over previous
"""Trainium2 Bass kernel for GroupLinear:
    out = einsum('lgi,lgj,ogij->lo', x1, x2, W.reshape(O,g,b,b)) + bias

Equivalent to Y = outer @ W.T + b where outer[l, k] (k = g*b*b + i*b + j) is
the blockwise outer product x1[l,g,i]*x2[l,g,j] -- a [2048, 65536] @
[65536, 1024] matmul whose LHS is generated on the fly.

Sharding: tensor-parallel over the contraction dim. Core c owns weight
blocks g in {2c, 2c+1} (K_local = 8192), computes a full [2048, 1024]
partial, and the host sums the 8 partials (+ bias). The replicated-x1
operand layout is prepared host-side (a pure gather -- no FLOPs); the
outer products themselves and all matmul FLOPs run on device (DVE + PE).

Schedule (per core): the kernel is PE-bound (~437us of matmul streaming),
so the schedule exists to keep the PE dense from ~9us on:
  - Phase 1 (l-blocks 0-2, chunk-outer): 6 psum tiles accumulate three
    l-blocks at once, so each W chunk is consumed 6 matmuls at a time as
    it arrives -- the PE's W demand rate (~280 GB/s) stays under the DMA
    supply rate while the 16.75 MB weight shard streams in.
  - Phase 2 (l-blocks 3-15, lb-outer): W is SBUF-resident; per chunk
    LDW + 2 matmuls (N=512) into a rotating set of psum banks.
  - Startup DMAs are split fine-grained and spread over both HWDGE
    queues (sync + scalar) so the first matmul issues as early as
    possible; output is bf16 to shrink the drain tail.
"""

import sys
import numpy as np

sys.path.insert(0, "/opt/trn_rl_repo")

import ml_dtypes  # noqa: E402

BF16 = ml_dtypes.bfloat16

L = 2048
H = 1024
O = 1024
B = 64
G = 16
NCORES = 8
GPC = G // NCORES          # weight blocks per core = 2
KL = GPC * B * B           # local contraction dim = 8192
NCHUNK = KL // 128         # 64 k-chunks of 128
LB = 128                   # l-block (tokens per psum tile)
NLB = L // LB              # 16
R1 = 3                     # l-blocks processed chunk-outer in phase 1

_cache = {}


def _build_nc():
    from concourse import bass, tile, bacc

    mybir = bass.mybir
    bf = mybir.dt.bfloat16
    f32 = mybir.dt.float32

    nc = bacc.Bacc("TRN2", target_bir_lowering=False, debug=False)
    wp = nc.dram_tensor("wp", [128, NCHUNK * O], bf, kind="ExternalInput")
    x1r = nc.dram_tensor("x1r", [NLB, 128, KL], bf, kind="ExternalInput")
    x2s = nc.dram_tensor("x2s", [GPC, 128, L], bf, kind="ExternalInput")
    out = nc.dram_tensor("out", [L, O], bf, kind="ExternalOutput")

    XA = R1 * LB               # x2 columns needed by phase 1 = 384

    with tile.TileContext(nc) as tc:
        with (
            tc.tile_pool(name="wpool", bufs=1) as wpool,
            tc.tile_pool(name="x2pool", bufs=1) as x2pool,
            tc.tile_pool(name="xpool", bufs=4) as xpool,
            tc.tile_pool(name="opool", bufs=2) as opool,
            tc.tile_pool(name="psum", bufs=2, space="PSUM") as psum,
        ):
            wt = wpool.tile([128, NCHUNK * O], bf)
            x2a = [x2pool.tile([128, XA], bf, tag=f"x2a_{g}", name=f"x2a_{g}") for g in range(GPC)]
            x2b = [x2pool.tile([128, L - XA], bf, tag=f"x2b_{g}", name=f"x2b_{g}") for g in range(GPC)]
            xts = [xpool.tile([128, KL], bf, tag="xt", name=f"xt_{r}") for r in range(R1)]

            def wpiece(c0, c1, eng):
                sl = slice(c0 * O, c1 * O)
                eng.dma_start(wt[:, sl], wp[:, sl])

            # startup streams, in consumption order, small pieces first.
            # sync carries the weight stream (semaphore granularity must
            # track the PE's chunk-by-chunk consumption in phase 1);
            # scalar carries x2 + the phase-1 slab pieces.
            W_PIECES = [(0, 1), (1, 2), (2, 3), (3, 4), (4, 6), (6, 8)] + [
                (a, a + 4) for a in range(8, NCHUNK, 4)
            ]
            SLAB_PIECES = [(0, 512), (512, 2048), (2048, 4096), (4096, KL)]
            for c0, c1 in W_PIECES:
                wpiece(c0, c1, nc.sync)
            # scalar leads with exactly what the first DVE mul needs -- the
            # DMA engines ramp slowly for the first ~5us, so order = need.
            nc.scalar.dma_start(xts[0][:, 0:512], x1r[0][:, 0:512])
            for g in range(GPC):
                nc.scalar.dma_start(x2a[g][:], x2s[g][:, 0:XA])
            for r in range(1, R1):
                nc.scalar.dma_start(xts[r][:, 0:512], x1r[r][:, 0:512])
            for a, b_ in SLAB_PIECES[1:]:
                for r in range(R1):
                    nc.scalar.dma_start(xts[r][:, a:b_], x1r[r][:, a:b_])
            for g in range(GPC):
                nc.scalar.dma_start(x2b[g][:], x2s[g][:, XA:L])

            # ---- phase 1: l-blocks 0..R1-1, chunk-outer ----
            ps1 = [
                [psum.tile([128, 512], f32, name=f"psA{r}{h}", tag=f"psA{r}{h}",
                           bufs=1) for h in range(2)]
                for r in range(R1)
            ]
            for c in range(NCHUNK):
                g = c >> 5
                csl = slice(c * 128, (c + 1) * 128)
                first, last = c == 0, c == NCHUNK - 1
                for r in range(R1):
                    nc.vector.tensor_mul(
                        xts[r][:, csl], xts[r][:, csl],
                        x2a[g][:, r * LB:(r + 1) * LB],
                    )
                for r in range(R1):
                    for h in range(2):
                        nc.tensor.matmul(
                            ps1[r][h][:],
                            xts[r][:, csl],
                            wt[:, c * O + h * 512:c * O + (h + 1) * 512],
                            start=first,
                            stop=last,
                        )
            xt_next = xpool.tile([128, KL], bf, tag="xt", name="xt")
            nc.scalar.dma_start(xt_next[:], x1r[R1])

            for r in range(R1):
                ot = opool.tile([128, O], bf, tag="ot", name="ot")
                nc.scalar.mul(ot[:, 0:512], ps1[r][0][:], 1.0)
                nc.scalar.mul(ot[:, 512:O], ps1[r][1][:], 1.0)
                nc.sync.dma_start(out[r * LB:(r + 1) * LB, :], ot[:])

            # ---- phase 2: l-blocks R1..NLB-1, lb-outer, two-pass over the
            # psum halves (ps0's drain overlaps ps1's matmul pass) ----
            for lb in range(R1, NLB):
                xt = xt_next
                if lb + 1 < NLB:
                    xt_next = xpool.tile([128, KL], bf, tag="xt", name="xt")
                    nc.scalar.dma_start(xt_next[:], x1r[lb + 1])
                lsl = slice(lb * LB, (lb + 1) * LB)
                bsl = slice(lb * LB - XA, (lb + 1) * LB - XA)
                ps0 = psum.tile([128, 512], f32, tag="ps", name="ps0")
                ps1_ = psum.tile([128, 512], f32, tag="ps", name="ps1")
                for c in range(NCHUNK):
                    g = c >> 5
                    csl = slice(c * 128, (c + 1) * 128)
                    nc.vector.tensor_mul(xt[:, csl], xt[:, csl], x2b[g][:, bsl])
                    nc.tensor.matmul(
                        ps0[:], xt[:, csl], wt[:, c * O:c * O + 512],
                        start=(c == 0), stop=(c == NCHUNK - 1),
                    )
                if lb == NLB - 1:
                    # separate half tiles: each half's output DMA depends
                    # only on its own ACT, so half 0 ships mid-block
                    oh0 = opool.tile([128, 512], bf, tag="oh0", name="oh0", bufs=1)
                    oh1 = opool.tile([128, 512], bf, tag="oh1", name="oh1", bufs=1)
                    nc.scalar.mul(oh0[:], ps0[:], 1.0)
                    nc.sync.dma_start(out[lsl, 0:512], oh0[:])
                else:
                    ot = opool.tile([128, O], bf, tag="ot", name="ot")
                    nc.scalar.mul(ot[:, 0:512], ps0[:], 1.0)
                for c in range(NCHUNK):
                    csl = slice(c * 128, (c + 1) * 128)
                    nc.tensor.matmul(
                        ps1_[:], xt[:, csl], wt[:, c * O + 512:(c + 1) * O],
                        start=(c == 0), stop=(c == NCHUNK - 1),
                    )
                if lb == NLB - 1:
                    nc.scalar.mul(oh1[:], ps1_[:], 1.0)
                    nc.sync.dma_start(out[lsl, 512:O], oh1[:])
                else:
                    nc.scalar.mul(ot[:, 512:O], ps1_[:], 1.0)
                    nc.sync.dma_start(out[lsl, :], ot[:])

    nc.compile()
    return nc


def _prep_inputs(input1, input2, W):
    """Host-side shard + layout (transposes / gathers / dtype casts only)."""
    x1 = np.ascontiguousarray(input1, dtype=np.float32)
    x2 = np.ascontiguousarray(input2, dtype=np.float32)
    Wt = np.ascontiguousarray(W.T, dtype=np.float32)  # [65536, 1024], k-major

    in_maps = []
    for core in range(NCORES):
        ks = slice(core * KL, (core + 1) * KL)
        gs = slice(core * GPC, (core + 1) * GPC)
        # weights: [k_local, o] -> [c, p, o] -> [p, c*O + o]
        wp = (
            Wt[ks]
            .reshape(NCHUNK, 128, O)
            .transpose(1, 0, 2)
            .reshape(128, NCHUNK * O)
            .astype(BF16)
        )
        # x1 replicated over j: k_local = g*B*B + i*B + j -> x1[l, g, i]
        x1g = x1.reshape(L, G, B)[:, gs, :].transpose(1, 2, 0)  # [g, i, l]
        rep = np.repeat(x1g, B, axis=1).reshape(KL, L)          # [k_local, l]
        x1r = (
            rep.reshape(NCHUNK, 128, NLB, LB)
            .transpose(2, 1, 0, 3)
            .reshape(NLB, 128, KL)
            .astype(BF16)
        )
        # x2 stacked twice along partitions: row p -> j = p % 64
        x2g = x2.reshape(L, G, B)[:, gs, :].transpose(1, 2, 0)  # [g, j, l]
        x2st = np.concatenate([x2g, x2g], axis=1).astype(BF16)  # [g, 128, l]
        in_maps.append(
            {
                "wp": np.ascontiguousarray(wp),
                "x1r": np.ascontiguousarray(x1r),
                "x2s": np.ascontiguousarray(x2st),
            }
        )
    return in_maps


def run(input1, input2, W, b, trace=False, tmpdir=None):
    """Shard, run on 8 NeuronCores, unshard. Returns (out, BassKernelResults)."""
    from concourse.bass_utils import run_bass_kernel_spmd

    if "nc" not in _cache:
        _cache["nc"] = _build_nc()
    nc = _cache["nc"]

    in_maps = _prep_inputs(input1, input2, W)
    res = run_bass_kernel_spmd(
        nc, in_maps, list(range(NCORES)), trace=trace, tmpdir=tmpdir
    )
    acc = np.zeros((L, O), dtype=np.float32)
    for core in range(NCORES):
        acc += res.results[core]["out"].astype(np.float32)
    acc += np.asarray(b, dtype=np.float32)[None, :]
    return acc, res


def kernel(input1, input2, W, b):
    out, _ = run(input1, input2, W, b, trace=False)
    return out


if __name__ == "__main__":
    rng = np.random.default_rng(0)
    x1 = rng.standard_normal((L, H), dtype=np.float32)
    x2 = rng.standard_normal((L, H), dtype=np.float32)
    W = rng.standard_normal((O, H * B), dtype=np.float32) / 256.0
    b = rng.standard_normal((O,), dtype=np.float32) / 256.0
    out = kernel(x1, x2, W, b)
    print("out", out.shape, out.dtype, float(np.abs(out).max()))


# revision 9
# speedup vs baseline: 1.0727x; 1.0015x over previous
"""Trainium2 Bass kernel for GroupLinear:
    out = einsum('lgi,lgj,ogij->lo', x1, x2, W.reshape(O,g,b,b)) + bias

Equivalent to Y = outer @ W.T + b where outer[l, k] (k = g*b*b + i*b + j) is
the blockwise outer product x1[l,g,i]*x2[l,g,j] -- a [2048, 65536] @
[65536, 1024] matmul whose LHS is generated on the fly.

Sharding: tensor-parallel over the contraction dim. Core c owns weight
blocks g in {2c, 2c+1} (K_local = 8192), computes a full [2048, 1024]
partial, and the host sums the 8 partials (+ bias). The replicated-x1
operand layout is prepared host-side (a pure gather -- no FLOPs); the
outer products themselves and all matmul FLOPs run on device (DVE + PE).

Schedule (per core): the kernel is PE-bound (~437us of matmul streaming),
so the schedule exists to keep the PE dense from ~9us on:
  - Phase 1 (l-blocks 0-2, chunk-outer): 6 psum tiles accumulate three
    l-blocks at once, so each W chunk is consumed 6 matmuls at a time as
    it arrives -- the PE's W demand rate (~280 GB/s) stays under the DMA
    supply rate while the 16.75 MB weight shard streams in.
  - Phase 2 (l-blocks 3-15, lb-outer): W is SBUF-resident; per chunk
    LDW + 2 matmuls (N=512) into a rotating set of psum banks.
  - Startup DMAs are split fine-grained and spread over both HWDGE
    queues (sync + scalar) so the first matmul issues as early as
    possible; output is bf16 to shrink the drain tail.
"""

import sys
import numpy as np

sys.path.insert(0, "/opt/trn_rl_repo")

import ml_dtypes  # noqa: E402

BF16 = ml_dtypes.bfloat16

L = 2048
H = 1024
O = 1024
B = 64
G = 16
NCORES = 8
GPC = G // NCORES          # weight blocks per core = 2
KL = GPC * B * B           # local contraction dim = 8192
NCHUNK = KL // 128         # 64 k-chunks of 128
LB = 128                   # l-block (tokens per psum tile)
NLB = L // LB              # 16
R1 = 3                     # l-blocks processed chunk-outer in phase 1

_cache = {}


def _build_nc():
    from concourse import bass, tile, bacc

    mybir = bass.mybir
    bf = mybir.dt.bfloat16
    f32 = mybir.dt.float32

    nc = bacc.Bacc("TRN2", target_bir_lowering=False, debug=False)
    XA = R1 * LB               # x2 columns needed by phase 1 = 384
    HD = 3 * 512 + 2 * XA      # head blob: 3 slab-heads + 2 x2 slices
    wp = nc.dram_tensor("wp", [128, NCHUNK * O], bf, kind="ExternalInput")
    x1r = nc.dram_tensor("x1r", [NLB, 128, KL], bf, kind="ExternalInput")
    x2s = nc.dram_tensor("x2s", [GPC, 128, L], bf, kind="ExternalInput")
    hd = nc.dram_tensor("hd", [128, HD], bf, kind="ExternalInput")
    out = nc.dram_tensor("out", [L, O], bf, kind="ExternalOutput")

    with tile.TileContext(nc) as tc:
        with (
            tc.tile_pool(name="wpool", bufs=1) as wpool,
            tc.tile_pool(name="x2pool", bufs=1) as x2pool,
            tc.tile_pool(name="xpool", bufs=4) as xpool,
            tc.tile_pool(name="opool", bufs=2) as opool,
            tc.tile_pool(name="psum", bufs=2, space="PSUM") as psum,
        ):
            wt = wpool.tile([128, NCHUNK * O], bf)
            head = x2pool.tile([128, HD], bf, tag="head", name="head")
            x2b = [x2pool.tile([128, L - XA], bf, tag=f"x2b_{g}", name=f"x2b_{g}") for g in range(GPC)]
            xts = [xpool.tile([128, KL], bf, tag="xt", name=f"xt_{r}") for r in range(R1)]

            def xa(g):           # x2 slice [128, XA] for group g inside head
                return head[:, 3 * 512 + g * XA:3 * 512 + (g + 1) * XA]

            def xslab(r, csl):   # slab r cols csl, head-resident for c<4
                if csl.stop <= 512:
                    return head[:, r * 512 + csl.start:r * 512 + csl.stop]
                return xts[r][:, csl]

            def wpiece(c0, c1, eng):
                sl = slice(c0 * O, c1 * O)
                eng.dma_start(wt[:, sl], wp[:, sl])

            # startup streams, in consumption order, small pieces first.
            # sync carries the weight stream (semaphore granularity must
            # track the PE's chunk-by-chunk consumption in phase 1);
            # scalar carries x2 + the phase-1 slab pieces.
            W_PIECES = [(0, 1), (1, 4)] + [(a, a + 4) for a in range(4, NCHUNK, 4)]
            SLAB_PIECES = [(512, 2048), (2048, 4096), (4096, KL)]
            for c0, c1 in W_PIECES:
                wpiece(c0, c1, nc.sync)
            # scalar leads with the head blob: one 128-packet transfer that
            # carries everything the first 4 chunks of all 3 l-blocks need
            # (the DMA path is packet-latency-bound for the first ~8us).
            nc.scalar.dma_start(head[:], hd[:])
            for a, b_ in SLAB_PIECES:
                for r in range(R1):
                    nc.scalar.dma_start(xts[r][:, a:b_], x1r[r][:, a:b_])
            for g in range(GPC):
                nc.scalar.dma_start(x2b[g][:], x2s[g][:, XA:L])

            # ---- phase 1: l-blocks 0..R1-1, chunk-outer ----
            ps1 = [
                [psum.tile([128, 512], f32, name=f"psA{r}{h}", tag=f"psA{r}{h}",
                           bufs=1) for h in range(2)]
                for r in range(R1)
            ]
            for c in range(NCHUNK):
                g = c >> 5
                csl = slice(c * 128, (c + 1) * 128)
                first, last = c == 0, c == NCHUNK - 1
                for r in range(R1):
                    t_ = xslab(r, csl)
                    nc.vector.tensor_mul(
                        t_, t_, xa(g)[:, r * LB:(r + 1) * LB],
                    )
                for r in range(R1):
                    for h in range(2):
                        nc.tensor.matmul(
                            ps1[r][h][:],
                            xslab(r, csl),
                            wt[:, c * O + h * 512:c * O + (h + 1) * 512],
                            start=first,
                            stop=last,
                        )
            xt_next = xpool.tile([128, KL], bf, tag="xt", name="xt")
            nc.scalar.dma_start(xt_next[:], x1r[R1])

            for r in range(R1):
                ot = opool.tile([128, O], bf, tag="ot", name="ot", bufs=1)
                nc.scalar.mul(ot[:, 0:512], ps1[r][0][:], 1.0)
                nc.scalar.mul(ot[:, 512:O], ps1[r][1][:], 1.0)
                nc.sync.dma_start(out[r * LB:(r + 1) * LB, :], ot[:])

            # ---- phase 2: l-blocks R1..NLB-1, lb-outer, two-pass over the
            # psum halves (ps0's drain overlaps ps1's matmul pass) ----
            for lb in range(R1, NLB):
                xt = xt_next
                if lb + 1 < NLB:
                    xt_next = xpool.tile([128, KL], bf, tag="xt", name="xt")
                    nc.scalar.dma_start(xt_next[:], x1r[lb + 1])
                lsl = slice(lb * LB, (lb + 1) * LB)
                bsl = slice(lb * LB - XA, (lb + 1) * LB - XA)
                ps0 = psum.tile([128, 512], f32, tag="ps", name="ps0")
                ps1_ = psum.tile([128, 512], f32, tag="ps", name="ps1")
                for c in range(NCHUNK):
                    g = c >> 5
                    csl = slice(c * 128, (c + 1) * 128)
                    nc.vector.tensor_mul(xt[:, csl], xt[:, csl], x2b[g][:, bsl])
                    nc.tensor.matmul(
                        ps0[:], xt[:, csl], wt[:, c * O:c * O + 512],
                        start=(c == 0), stop=(c == NCHUNK - 1),
                    )
                if lb == NLB - 1:
                    # separate half tiles: each half's output DMA depends
                    # only on its own ACT, so half 0 ships mid-block
                    oh0 = opool.tile([128, 512], bf, tag="oh0", name="oh0", bufs=1)
                    oh1 = opool.tile([128, 512], bf, tag="oh1", name="oh1", bufs=1)
                    nc.scalar.mul(oh0[:], ps0[:], 1.0)
                    nc.sync.dma_start(out[lsl, 0:512], oh0[:])
                else:
                    ot = opool.tile([128, O], bf, tag="ot", name="ot", bufs=1)
                    nc.scalar.mul(ot[:, 0:512], ps0[:], 1.0)
                for c in range(NCHUNK):
                    csl = slice(c * 128, (c + 1) * 128)
                    nc.tensor.matmul(
                        ps1_[:], xt[:, csl], wt[:, c * O + 512:(c + 1) * O],
                        start=(c == 0), stop=(c == NCHUNK - 1),
                    )
                if lb == NLB - 1:
                    nc.scalar.mul(oh1[:], ps1_[:], 1.0)
                    nc.sync.dma_start(out[lsl, 512:O], oh1[:])
                else:
                    nc.scalar.mul(ot[:, 512:O], ps1_[:], 1.0)
                    nc.sync.dma_start(out[lsl, :], ot[:])

    nc.compile()
    return nc


def _prep_inputs(input1, input2, W):
    """Host-side shard + layout (transposes / gathers / dtype casts only)."""
    x1 = np.ascontiguousarray(input1, dtype=np.float32)
    x2 = np.ascontiguousarray(input2, dtype=np.float32)
    Wt = np.ascontiguousarray(W.T, dtype=np.float32)  # [65536, 1024], k-major

    in_maps = []
    for core in range(NCORES):
        ks = slice(core * KL, (core + 1) * KL)
        gs = slice(core * GPC, (core + 1) * GPC)
        # weights: [k_local, o] -> [c, p, o] -> [p, c*O + o]
        wp = (
            Wt[ks]
            .reshape(NCHUNK, 128, O)
            .transpose(1, 0, 2)
            .reshape(128, NCHUNK * O)
            .astype(BF16)
        )
        # x1 replicated over j: k_local = g*B*B + i*B + j -> x1[l, g, i]
        x1g = x1.reshape(L, G, B)[:, gs, :].transpose(1, 2, 0)  # [g, i, l]
        rep = np.repeat(x1g, B, axis=1).reshape(KL, L)          # [k_local, l]
        x1r = (
            rep.reshape(NCHUNK, 128, NLB, LB)
            .transpose(2, 1, 0, 3)
            .reshape(NLB, 128, KL)
            .astype(BF16)
        )
        # x2 stacked twice along partitions: row p -> j = p % 64
        x2g = x2.reshape(L, G, B)[:, gs, :].transpose(1, 2, 0)  # [g, j, l]
        x2st = np.concatenate([x2g, x2g], axis=1).astype(BF16)  # [g, 128, l]
        hd = np.concatenate(
            [x1r[r, :, 0:512] for r in range(R1)]
            + [x2st[g][:, 0:R1 * 128] for g in range(GPC)],
            axis=1,
        )
        in_maps.append(
            {
                "wp": np.ascontiguousarray(wp),
                "x1r": np.ascontiguousarray(x1r),
                "x2s": np.ascontiguousarray(x2st),
                "hd": np.ascontiguousarray(hd),
            }
        )
    return in_maps


def run(input1, input2, W, b, trace=False, tmpdir=None):
    """Shard, run on 8 NeuronCores, unshard. Returns (out, BassKernelResults)."""
    from concourse.bass_utils import run_bass_kernel_spmd

    if "nc" not in _cache:
        _cache["nc"] = _build_nc()
    nc = _cache["nc"]

    in_maps = _prep_inputs(input1, input2, W)
    res = run_bass_kernel_spmd(
        nc, in_maps, list(range(NCORES)), trace=trace, tmpdir=tmpdir
    )
    acc = np.zeros((L, O), dtype=np.float32)
    for core in range(NCORES):
        acc += res.results[core]["out"].astype(np.float32)
    acc += np.asarray(b, dtype=np.float32)[None, :]
    return acc, res


def kernel(input1, input2, W, b):
    out, _ = run(input1, input2, W, b, trace=False)
    return out


if __name__ == "__main__":
    rng = np.random.default_rng(0)
    x1 = rng.standard_normal((L, H), dtype=np.float32)
    x2 = rng.standard_normal((L, H), dtype=np.float32)
    W = rng.standard_normal((O, H * B), dtype=np.float32) / 256.0
    b = rng.standard_normal((O,), dtype=np.float32) / 256.0
    out = kernel(x1, x2, W, b)
    print("out", out.shape, out.dtype, float(np.abs(out).max()))


# revision 11
# speedup vs baseline: 1.0828x; 1.0094x over previous
"""Trainium2 Bass kernel for GroupLinear:
    out = einsum('lgi,lgj,ogij->lo', x1, x2, W.reshape(O,g,b,b)) + bias

Equivalent to Y = outer @ W.T + b where outer[l, k] (k = g*b*b + i*b + j) is
the blockwise outer product x1[l,g,i]*x2[l,g,j] -- a [2048, 65536] @
[65536, 1024] matmul whose LHS is generated on the fly.

Sharding: tensor-parallel over the contraction dim. Core c owns weight
blocks g in {2c, 2c+1} (K_local = 8192), computes a full [2048, 1024]
partial, and the host sums the 8 partials (+ bias). The replicated-x1
operand layout is prepared host-side (a pure gather -- no FLOPs); the
outer products themselves and all matmul FLOPs run on device (DVE + PE).

Schedule (per core): the kernel is PE-bound (~437us of matmul streaming),
so the schedule exists to keep the PE dense from ~9us on:
  - Phase 1 (l-blocks 0-2, chunk-outer): 6 psum tiles accumulate three
    l-blocks at once, so each W chunk is consumed 6 matmuls at a time as
    it arrives -- the PE's W demand rate (~280 GB/s) stays under the DMA
    supply rate while the 16.75 MB weight shard streams in.
  - Phase 2 (l-blocks 3-15, lb-outer): W is SBUF-resident; per chunk
    LDW + 2 matmuls (N=512) into a rotating set of psum banks.
  - Startup DMAs are split fine-grained and spread over both HWDGE
    queues (sync + scalar) so the first matmul issues as early as
    possible; output is bf16 to shrink the drain tail.
"""

import sys
import numpy as np

sys.path.insert(0, "/opt/trn_rl_repo")

import ml_dtypes  # noqa: E402

BF16 = ml_dtypes.bfloat16

L = 2048
H = 1024
O = 1024
B = 64
G = 16
NCORES = 8
GPC = G // NCORES          # weight blocks per core = 2
KL = GPC * B * B           # local contraction dim = 8192
NCHUNK = KL // 128         # 64 k-chunks of 128
LB = 128                   # l-block (tokens per psum tile)
NLB = L // LB              # 16
R1 = 3                     # l-blocks processed chunk-outer in phase 1

_cache = {}


def _build_nc():
    from concourse import bass, tile, bacc

    mybir = bass.mybir
    bf = mybir.dt.bfloat16
    f32 = mybir.dt.float32

    nc = bacc.Bacc("TRN2", target_bir_lowering=False, debug=False)
    XA = R1 * LB               # x2 columns needed by phase 1 = 384
    HDC = 640                  # head covers chunks 0-4 of each slab
    HD = 3 * HDC + 2 * XA      # head blob: 3 slab-heads + 2 x2 slices
    wp = nc.dram_tensor("wp", [128, NCHUNK * O], bf, kind="ExternalInput")
    x1r = nc.dram_tensor("x1r", [NLB, 128, KL], bf, kind="ExternalInput")
    x2s = nc.dram_tensor("x2s", [GPC, 128, L], bf, kind="ExternalInput")
    hd = nc.dram_tensor("hd", [128, HD], bf, kind="ExternalInput")
    out = nc.dram_tensor("out", [L, O], bf, kind="ExternalOutput")

    with tile.TileContext(nc) as tc:
        with (
            tc.tile_pool(name="wpool", bufs=1) as wpool,
            tc.tile_pool(name="x2pool", bufs=1) as x2pool,
            tc.tile_pool(name="xpool", bufs=4) as xpool,
            tc.tile_pool(name="opool", bufs=2) as opool,
            tc.tile_pool(name="psum", bufs=2, space="PSUM") as psum,
        ):
            wt = wpool.tile([128, NCHUNK * O], bf)
            head = x2pool.tile([128, HD], bf, tag="head", name="head")
            x2b = [x2pool.tile([128, L - XA], bf, tag=f"x2b_{g}", name=f"x2b_{g}") for g in range(GPC)]
            xts = [xpool.tile([128, KL], bf, tag="xt", name=f"xt_{r}") for r in range(R1)]

            def xa(g):           # x2 slice [128, XA] for group g inside head
                return head[:, 3 * HDC + g * XA:3 * HDC + (g + 1) * XA]

            def xslab(r, csl):   # slab r cols csl, head-resident early chunks
                if csl.stop <= HDC:
                    return head[:, r * HDC + csl.start:r * HDC + csl.stop]
                return xts[r][:, csl]

            def wpiece(c0, c1, eng):
                sl = slice(c0 * O, c1 * O)
                eng.dma_start(wt[:, sl], wp[:, sl])

            # startup streams, in consumption order, small pieces first.
            # sync carries the weight stream (semaphore granularity must
            # track the PE's chunk-by-chunk consumption in phase 1);
            # scalar carries x2 + the phase-1 slab pieces.
            W_PIECES = [(0, 1), (1, 2), (2, 4), (4, 6), (6, 8), (8, 10),
                        (10, 12), (12, 16)] + [
                (a, a + 4) for a in range(16, NCHUNK, 4)
            ]
            SLAB_PIECES = [(HDC, 2048), (2048, 4096), (4096, KL)]
            for c0, c1 in W_PIECES:
                wpiece(c0, c1, nc.sync)
            # scalar leads with the head blob: one 128-packet transfer that
            # carries everything the first 4 chunks of all 3 l-blocks need
            # (the DMA path is packet-latency-bound for the first ~8us).
            nc.scalar.dma_start(head[:], hd[:])
            for a, b_ in SLAB_PIECES:
                for r in range(R1):
                    nc.scalar.dma_start(xts[r][:, a:b_], x1r[r][:, a:b_])
            for g in range(GPC):
                nc.scalar.dma_start(x2b[g][:], x2s[g][:, XA:L])

            # ---- phase 1: l-blocks 0..R1-1, chunk-outer ----
            ps1 = [
                [psum.tile([128, 512], f32, name=f"psA{r}{h}", tag=f"psA{r}{h}",
                           bufs=1) for h in range(2)]
                for r in range(R1)
            ]
            for c in range(NCHUNK):
                g = c >> 5
                csl = slice(c * 128, (c + 1) * 128)
                first, last = c == 0, c == NCHUNK - 1
                for r in range(R1):
                    t_ = xslab(r, csl)
                    nc.vector.tensor_mul(
                        t_, t_, xa(g)[:, r * LB:(r + 1) * LB],
                    )
                for r in range(R1):
                    for h in range(2):
                        nc.tensor.matmul(
                            ps1[r][h][:],
                            xslab(r, csl),
                            wt[:, c * O + h * 512:c * O + (h + 1) * 512],
                            start=first,
                            stop=last,
                        )
            xt_next = xpool.tile([128, KL], bf, tag="xt", name="xt")
            nc.scalar.dma_start(xt_next[:], x1r[R1])

            for r in range(R1):
                ot = opool.tile([128, O], bf, tag="ot", name="ot", bufs=1)
                nc.scalar.mul(ot[:, 0:512], ps1[r][0][:], 1.0)
                nc.scalar.mul(ot[:, 512:O], ps1[r][1][:], 1.0)
                nc.sync.dma_start(out[r * LB:(r + 1) * LB, :], ot[:])

            # ---- phase 2: l-blocks R1..NLB-1, lb-outer, two-pass over the
            # psum halves (ps0's drain overlaps ps1's matmul pass) ----
            for lb in range(R1, NLB):
                xt = xt_next
                if lb + 1 < NLB:
                    xt_next = xpool.tile([128, KL], bf, tag="xt", name="xt")
                    nc.scalar.dma_start(xt_next[:], x1r[lb + 1])
                lsl = slice(lb * LB, (lb + 1) * LB)
                bsl = slice(lb * LB - XA, (lb + 1) * LB - XA)
                ps0 = psum.tile([128, 512], f32, tag="ps", name="ps0")
                ps1_ = psum.tile([128, 512], f32, tag="ps", name="ps1")
                for c in range(NCHUNK):
                    g = c >> 5
                    csl = slice(c * 128, (c + 1) * 128)
                    nc.vector.tensor_mul(xt[:, csl], xt[:, csl], x2b[g][:, bsl])
                    nc.tensor.matmul(
                        ps0[:], xt[:, csl], wt[:, c * O:c * O + 512],
                        start=(c == 0), stop=(c == NCHUNK - 1),
                    )
                if lb == NLB - 1:
                    # last block: drain in shrinking pieces so each output
                    # DMA (its own tile -> its own dep) ships while the next
                    # narrower accumulation pass still runs
                    oh0 = opool.tile([128, 512], bf, tag="oh0", name="oh0", bufs=1)
                    nc.scalar.mul(oh0[:], ps0[:], 1.0)
                    nc.sync.dma_start(out[lsl, 0:512], oh0[:])
                    for q, (q0, q1) in enumerate([(512, 768), (768, O)]):
                        psq = psum.tile([128, q1 - q0], f32, tag="ps",
                                        name="psq")
                        for c in range(NCHUNK):
                            csl = slice(c * 128, (c + 1) * 128)
                            nc.tensor.matmul(
                                psq[:], xt[:, csl], wt[:, c * O + q0:c * O + q1],
                                start=(c == 0), stop=(c == NCHUNK - 1),
                            )
                        oq = opool.tile([128, q1 - q0], bf, tag=f"oq{q}",
                                        name=f"oq{q}", bufs=1)
                        nc.scalar.mul(oq[:], psq[:], 1.0)
                        nc.sync.dma_start(out[lsl, q0:q1], oq[:])
                else:
                    ot = opool.tile([128, O], bf, tag="ot", name="ot", bufs=1)
                    nc.scalar.mul(ot[:, 0:512], ps0[:], 1.0)
                    for c in range(NCHUNK):
                        csl = slice(c * 128, (c + 1) * 128)
                        nc.tensor.matmul(
                            ps1_[:], xt[:, csl], wt[:, c * O + 512:(c + 1) * O],
                            start=(c == 0), stop=(c == NCHUNK - 1),
                        )
                    nc.scalar.mul(ot[:, 512:O], ps1_[:], 1.0)
                    nc.sync.dma_start(out[lsl, :], ot[:])

    nc.compile()
    return nc


def _prep_inputs(input1, input2, W):
    """Host-side shard + layout (transposes / gathers / dtype casts only)."""
    x1 = np.ascontiguousarray(input1, dtype=np.float32)
    x2 = np.ascontiguousarray(input2, dtype=np.float32)
    Wt = np.ascontiguousarray(W.T, dtype=np.float32)  # [65536, 1024], k-major

    in_maps = []
    for core in range(NCORES):
        ks = slice(core * KL, (core + 1) * KL)
        gs = slice(core * GPC, (core + 1) * GPC)
        # weights: [k_local, o] -> [c, p, o] -> [p, c*O + o]
        wp = (
            Wt[ks]
            .reshape(NCHUNK, 128, O)
            .transpose(1, 0, 2)
            .reshape(128, NCHUNK * O)
            .astype(BF16)
        )
        # x1 replicated over j: k_local = g*B*B + i*B + j -> x1[l, g, i]
        x1g = x1.reshape(L, G, B)[:, gs, :].transpose(1, 2, 0)  # [g, i, l]
        rep = np.repeat(x1g, B, axis=1).reshape(KL, L)          # [k_local, l]
        x1r = (
            rep.reshape(NCHUNK, 128, NLB, LB)
            .transpose(2, 1, 0, 3)
            .reshape(NLB, 128, KL)
            .astype(BF16)
        )
        # x2 stacked twice along partitions: row p -> j = p % 64
        x2g = x2.reshape(L, G, B)[:, gs, :].transpose(1, 2, 0)  # [g, j, l]
        x2st = np.concatenate([x2g, x2g], axis=1).astype(BF16)  # [g, 128, l]
        hd = np.concatenate(
            [x1r[r, :, 0:640] for r in range(R1)]
            + [x2st[g][:, 0:R1 * 128] for g in range(GPC)],
            axis=1,
        )
        in_maps.append(
            {
                "wp": np.ascontiguousarray(wp),
                "x1r": np.ascontiguousarray(x1r),
                "x2s": np.ascontiguousarray(x2st),
                "hd": np.ascontiguousarray(hd),
            }
        )
    return in_maps


def run(input1, input2, W, b, trace=False, tmpdir=None):
    """Shard, run on 8 NeuronCores, unshard. Returns (out, BassKernelResults)."""
    from concourse.bass_utils import run_bass_kernel_spmd

    if "nc" not in _cache:
        _cache["nc"] = _build_nc()
    nc = _cache["nc"]

    in_maps = _prep_inputs(input1, input2, W)
    res = run_bass_kernel_spmd(
        nc, in_maps, list(range(NCORES)), trace=trace, tmpdir=tmpdir
    )
    acc = np.zeros((L, O), dtype=np.float32)
    for core in range(NCORES):
        acc += res.results[core]["out"].astype(np.float32)
    acc += np.asarray(b, dtype=np.float32)[None, :]
    return acc, res


def kernel(input1, input2, W, b):
    out, _ = run(input1, input2, W, b, trace=False)
    return out


if __name__ == "__main__":
    rng = np.random.default_rng(0)
    x1 = rng.standard_normal((L, H), dtype=np.float32)
    x2 = rng.standard_normal((L, H), dtype=np.float32)
    W = rng.standard_normal((O, H * B), dtype=np.float32) / 256.0
    b = rng.standard_normal((O,), dtype=np.float32) / 256.0
    out = kernel(x1, x2, W, b)
    print("out", out.shape, out.dtype, float(np.abs(out).max()))
